# revision 2
# baseline (speedup 1.0000x reference)
"""Trainium2 Bass kernel for nn_AdaMLP (MoE routing, 64 experts, 2-layer MLP).

Strategy: expert-parallel over 8 NeuronCores; core i owns experts
[8i, 8i+8). The host groups slots by expert (the MoE dispatch), pads
each group to capacity C, and ships per core:
  - the 8 experts' weights quantized to fp8 e3m4 with per-output-channel
    scales (layer-1 scales folded into layer-2 weights, layer-2 scales
    applied on the PSUM->SBUF output op), clip factor per channel chosen
    to minimize weight MSE,
  - transposed slot groups xT in fp16,
  - per-expert output scale/bias columns in f32.
Each core computes, per expert:  H^T = relu(W1q^T-blocks @ xT),
Out^T = s2 * (W2q-blocks @ H^T) + b2, with the fp8 weights as the
stationary matmul operand.  fp8 weights halve the dominant HBM weight
stream (8.4 MB -> 4.2 MB per core) vs bf16; rel err ~1.8e-2 vs the f32
reference (gate 2e-2), deterministic for a fixed input set.

Schedule (v2, from trace analysis of the merged-DMA baseline): ALL input
DMAs ride the Sync HWDGE ring in exact consumption order:
  xt, sb, [w1(e0)], [w2(e0)|w1(e1)], ..., [w2(e6)|w1(e7)], [w2(e7)]
The pair-merged weight DMAs mean L1(e) is gated only on bytes it
actually needs (w1(e) is the tail of its pair), unlike the baseline's
per-expert [w1|w2] merge where L1(e0) waited ~1.3us for w2(e0) bytes.
xt/sb at the ring head land during the DMA pipeline fill, so the first
matmul starts as soon as w1(e0) lands. 10 issues x ~0.4-0.7us
descriptor-gen on Sync stay ahead of the ~1.1us/512KB data pitch.
The expert loop is software-pipelined via tile_wait_until floors so the
in-order PE runs L1(e+1) while Vector relus expert e. Vector does only
relus; Scalar does the dequant Copies. The final expert's dequant+store
is split per-oc-chunk across engines (oc0: Scalar ACTIVATE + Sync
store; oc1: Vector mult + Scalar store) so the two halves overlap.
Remaining exec time is dominated by fixed NEFF protocol: ~5.3us
preamble (start barriers + iram loads) and ~7us teardown (codegen's
unconditional 256-semaphore reset storm, ~51 resets/engine, PE slowest
at ~115 cyc each) around ~13us of streaming body.
"""

import numpy as np

P = 128                    # SBUF partitions
DIM = 256                  # slot dim
R = 1024                   # hidden dim
E = 64                     # num experts
NCORES = 8
EPC = E // NCORES          # experts per core
DC = DIM // P              # layer-1 contraction chunks (2)
RC = R // P                # r chunks (8)
OC = DIM // P              # output dim chunks (2)
W1C = DC * R               # w1 columns per expert (2048)
W2C = RC * DIM             # w2 columns per expert (2048)
WCOLS = W1C + W2C          # weight columns per expert (4096)

# fp8 e3m4 weight storage roughly halves the (dominant) weight-table DMA
# traffic vs bf16; measured rel err ~1.8e-2 vs the f32 reference (inside
# the 2e-2 gate). Set False for the bf16 fallback (~3.3e-3).
USE_FP8 = True
SHRINK_SEMS = True

_GRAPH_CACHE: dict = {}


def _build_graph(C: int, use_fp8: bool):
    import concourse.bacc as bacc
    import concourse.bass as bass_mod
    import concourse.tile as tile
    from concourse import mybir

    # Shrink the kernel semaphore range (the walrus codegen epilogue's
    # 256-sem reset storm is NOT affected by this — it is unconditional —
    # but a compact kernel range keeps sem allocation stable).
    if SHRINK_SEMS:
        bass_mod.get_kernel_semaphore_range = lambda: range(150, 198)
        import concourse.bass_utils as bu

        if not getattr(bu.get_walrus_args, "_max_sem_patch", False):
            orig_gwa = bu.get_walrus_args

            def _gwa(*a, **kw):
                return [*orig_gwa(*a, **kw), "--max-sem-num=198"]

            _gwa._max_sem_patch = True
            bu.get_walrus_args = _gwa

    f32 = mybir.dt.float32
    wdt = mybir.dt.float8e3 if use_fp8 else mybir.dt.bfloat16
    xdt = mybir.dt.float16 if use_fp8 else mybir.dt.bfloat16

    mx = mybir.AluOpType.max
    mm = mybir.AluOpType.mult

    nc = bacc.Bacc(None, target_bir_lowering=False)
    xt_ext = nc.declare_dram_parameter("xt", [P, DC * EPC * C], xdt, isOutput=False)
    # flat weight layout: per partition row, [w1(e0)|w2(e0)|w1(e1)|...]
    wg_ext = nc.declare_dram_parameter("wg", [P, EPC * WCOLS], wdt, isOutput=False)
    # per-expert output scale+bias columns: [s2 | b2] per oc chunk
    sb_ext = nc.declare_dram_parameter("sb", [P, EPC * OC * 2], f32, isOutput=False)
    out_ext = nc.declare_dram_parameter("out", [P, EPC * OC * C], f32, isOutput=True)

    with tile.TileContext(nc) as tc:
        with (
            tc.tile_pool(name="xpool", bufs=1) as xpool,
            tc.tile_pool(name="wpool", bufs=1) as wpool,
            tc.tile_pool(name="hpool", bufs=4) as hpool,
            tc.tile_pool(name="opool", bufs=1) as opool,
            tc.tile_pool(name="ps1pool", bufs=3, space="PSUM") as ps1pool,
            tc.tile_pool(name="ps2pool", bufs=5, space="PSUM") as ps2pool,
        ):
            # The Sync HWDGE ring carries every input in consumption order.
            # Rings complete in order, so each DMA's semaphore gates exactly
            # the bytes scheduled before it; descriptor-gen (~0.4-0.7us per
            # issue) stays ahead of the ~1.1us/512KB data pitch.
            xt = xpool.tile([P, DC * EPC * C], xdt)
            nc.sync.dma_start(xt[:], xt_ext[:])
            sb = xpool.tile([P, EPC * OC * 2], f32)
            nc.sync.dma_start(sb[:], sb_ext[:])

            # Pair-merged weight DMAs with per-region tiles:
            #   wtile[0]   = w1(e0)                 cols [0, 2048)
            #   wtile[k]   = [w2(e[k-1]) | w1(e[k])] cols [4096k-2048, 4096k+2048)
            #   wtile[EPC] = w2(e7)                 cols [30720, 32768)
            # L1(e) reads the tail half of wtile[e]; L2(e) reads the head
            # half of wtile[e+1]; every matmul block lies in exactly one
            # tile, so Tile-framework deps gate on the right DMA.
            wtiles = []
            bounds = [0] + [k * WCOLS + W1C for k in range(EPC)] + [EPC * WCOLS]
            for k in range(EPC + 1):
                c0, c1 = bounds[k], bounds[k + 1]
                wt = wpool.tile([P, c1 - c0], wdt)
                nc.sync.dma_start(wt[:], wg_ext[:, c0:c1])
                wtiles.append(wt)

            def w1_slice(e, dc_i, rc_i):
                # w1(e) block (dc_i, rc_i) lives in wtile[e]; for e==0 the
                # tile starts at col 0, else at its second half (offset W2C).
                off = 0 if e == 0 else W2C
                c = off + dc_i * R + rc_i * P
                return wtiles[e][:, c : c + P]

            def w2_slice(e, rc_i, oc_i):
                # w2(e) block lives at the head of wtile[e+1].
                c = rc_i * DIM + oc_i * P
                return wtiles[e + 1][:, c : c + P]

            # single output staging tile; experts 0..6 stored in one bulk
            # DMA during expert 7's compute, expert 7 split per-oc across
            # Sync and Scalar so dequant and store of the halves overlap.
            out_sb = opool.tile([P, EPC * OC * C], f32)

            hs = {}

            def layer2(e):
                # layer 2: Out^T[dim,:] = sum_r W2[r, dim-block] . H^T[r, :]
                h = hs.pop(e)
                ps2 = ps2pool.tile([P, OC * C], f32)
                for oc_i in range(OC):
                    for rc_i in range(RC):
                        nc.tensor.matmul(
                            ps2[:, oc_i * C : oc_i * C + C],
                            w2_slice(e, rc_i, oc_i),
                            h[:, rc_i * C : rc_i * C + C],
                            start=(rc_i == 0),
                            stop=(rc_i == RC - 1),
                        )
                # dequant scale on Scalar only (activation Copy with
                # per-partition scale; b2 == 0, checked on host): Vector
                # must stay relu-only — giving it out-ops chains relu(e)
                # behind L2(e-1) through its in-order queue. GPSIMD can't
                # read PSUM.
                base = e * OC * C
                for oc_i in range(OC):
                    if e == EPC - 1 and oc_i == 1:
                        # last expert only: run oc1 on Vector in parallel
                        # with oc0 on Scalar — no later relu exists for this
                        # to block, and it halves the final dequant latency.
                        nc.vector.tensor_scalar(
                            out_sb[:, base + C : base + 2 * C],
                            ps2[:, C : 2 * C],
                            sb[:, (e * OC + 1) * 2 : (e * OC + 1) * 2 + 1],
                            None,
                            mm,
                        )
                        continue
                    nc.scalar.activation(
                        out_sb[:, base + oc_i * C : base + (oc_i + 1) * C],
                        ps2[:, oc_i * C : oc_i * C + C],
                        mybir.ActivationFunctionType.Copy,
                        bias=0.0,
                        scale=sb[:, (e * OC + oc_i) * 2 : (e * OC + oc_i) * 2 + 1],
                    )
                if e == EPC - 2:
                    # store experts 0..6 while expert 7 computes; only
                    # e7's store rides the critical tail.
                    nc.scalar.dma_start(
                        out_ext[:, : (EPC - 1) * OC * C],
                        out_sb[:, : (EPC - 1) * OC * C],
                    )
                if e == EPC - 1:
                    # split the final store: oc0 (Scalar ACTIVATE result)
                    # goes out on Sync — idle by now — while Vector still
                    # dequants oc1; oc1 follows on Scalar. Two ~350-cycle
                    # descriptor-gens in parallel instead of one ~650.
                    nc.sync.dma_start(
                        out_ext[:, base : base + C], out_sb[:, base : base + C]
                    )
                    nc.scalar.dma_start(
                        out_ext[:, base + C : base + 2 * C],
                        out_sb[:, base + C : base + 2 * C],
                    )

            # Software pipeline: the PE queue is in-order, so L1(e) must be
            # SCHEDULED before L2(e-1) — the PE then runs L1(e) while Vector
            # does relu(e-1) instead of stalling. Emission order alone does
            # not guarantee this (the Tile scheduler re-simulates and its
            # DMA model makes the weight arrivals look later than they
            # land), so pace the schedule explicitly: L1(e) floored at the
            # stream cadence, L2(e-1) floored just after L1(e).
            for e in range(EPC):
                with tc.tile_wait_until(0.010 + 0.003 * e):
                    # layer 1: H^T[r,:] = sum_d W1[d, r-block] . xT[d, :]
                    # 8 accumulation groups at offsets of one PSUM tile.
                    ps1 = ps1pool.tile([P, RC * C], f32)
                    for rc_i in range(RC):
                        for dc_i in range(DC):
                            nc.tensor.matmul(
                                ps1[:, rc_i * C : rc_i * C + C],
                                w1_slice(e, dc_i, rc_i),
                                xt[:, (dc_i * EPC + e) * C : (dc_i * EPC + e) * C + C],
                                start=(dc_i == 0),
                                stop=(dc_i == DC - 1),
                            )
                    # single fused relu over all 8 chunks (b1 == 0; checked
                    # on host), on Vector; Vector does nothing else.
                    h = hpool.tile([P, RC * C], xdt)
                    nc.vector.tensor_scalar(h[:], ps1[:], 0.0, None, mx)
                    hs[e] = h
                if e >= 1:
                    with tc.tile_wait_until(0.011 + 0.003 * e):
                        layer2(e - 1)
            with tc.tile_wait_until(0.011 + 0.003 * EPC):
                layer2(EPC - 1)
    nc.compile()
    return nc


def _get_graph(C: int, use_fp8: bool):
    key = (C, use_fp8)
    if key not in _GRAPH_CACHE:
        _GRAPH_CACHE[key] = _build_graph(C, use_fp8)
    return _GRAPH_CACHE[key]


def _quant_e3m4_chan(w, np_e3m4):
    """Quantize w [n_chan along last axis] to e3m4 with per-channel scale;
    clip factor per channel picked from a small grid to minimize MSE.
    w: (..., K, N) quantized per-column-N over axis -2. Returns (q, s)."""
    amax = np.abs(w).max(axis=-2, keepdims=True)
    amax = np.maximum(amax, 1e-30)
    best_err = None
    best_q = None
    best_s = None
    for g in (1.0, 1.05, 1.1, 1.2, 1.35, 1.5):
        s = amax * (g / 15.5)
        q = np.clip(w / s, -15.5, 15.5).astype(np_e3m4)
        err = ((q.astype(np.float32) * s - w) ** 2).sum(axis=-2, keepdims=True)
        if best_err is None:
            best_err, best_q, best_s = err, q, s
        else:
            m = err < best_err
            best_err = np.where(m, err, best_err)
            best_q = np.where(np.broadcast_to(m, q.shape), q, best_q)
            best_s = np.where(m, s, best_s)
    return best_q, best_s[..., 0, :]


def _run(inputs: dict, trace: bool = False, trace_cores=None, use_bf16=None,
         use_fp8=None, **spmd_kwargs):
    from concourse.bass_utils import run_bass_kernel_spmd
    import ml_dtypes

    if use_fp8 is None:
        use_fp8 = USE_FP8 and not use_bf16

    if use_fp8:
        wdt_np = ml_dtypes.float8_e3m4
        xdt_np = np.float16
    else:
        wdt_np = ml_dtypes.bfloat16
        xdt_np = ml_dtypes.bfloat16

    slots = np.asarray(inputs["slots"], np.float32)
    w1 = np.asarray(inputs["w1"], np.float32)
    b1 = np.asarray(inputs["b1"], np.float32)
    w2 = np.asarray(inputs["w2"], np.float32)
    b2 = np.asarray(inputs["b2"], np.float32)
    indices = np.asarray(inputs["indices"]).astype(np.int64)

    B, K, D = slots.shape
    assert D == DIM and w1.shape == (E, DIM, R) and w2.shape == (E, R, DIM)
    assert not b1.any(), "nonzero b1 needs the per-chunk bias path"
    assert not b2.any(), "nonzero b2 needs the tensor_scalar output path"
    X = slots.reshape(B * K, DIM)
    idx = indices.reshape(B * K)

    counts = np.bincount(idx, minlength=E)
    C = max(int(counts.max()), 16)
    C = ((C + 15) // 16) * 16  # stable capacities -> stable NEFF cache keys

    if use_fp8:
        # per-channel-r scales for w1; fold s1 into w2 rows; per-channel-d
        # scales for w2 applied on-device via the output tensor_scalar.
        w1q, s1 = _quant_e3m4_chan(w1, wdt_np)          # (E,D,R), (E,R)
        w2p = w2 * s1[:, :, None]
        w2q, s2 = _quant_e3m4_chan(w2p, wdt_np)          # (E,R,D), (E,D)
    else:
        w1q = w1.astype(wdt_np)
        w2q = w2.astype(wdt_np)
        s2 = np.ones((E, DIM), np.float32)

    in_maps = []
    pos_lists = []
    for core in range(NCORES):
        xt = np.zeros((P, DC * EPC * C), xdt_np)
        wg = np.empty((P, EPC * WCOLS), wdt_np)
        sb = np.zeros((P, EPC * OC * 2), np.float32)
        core_pos = []
        for e in range(EPC):
            g = core * EPC + e
            pos = np.nonzero(idx == g)[0]
            core_pos.append(pos)
            n = len(pos)
            if n:
                xeT = X[pos].T.astype(xdt_np)  # [DIM, n]
                for dc_i in range(DC):
                    xt[:, (dc_i * EPC + e) * C : (dc_i * EPC + e) * C + n] = (
                        xeT[dc_i * P : (dc_i + 1) * P]
                    )
            wg[:, e * WCOLS : e * WCOLS + W1C] = (
                w1q[g].reshape(DC, P, R).transpose(1, 0, 2).reshape(P, W1C)
            )
            wg[:, e * WCOLS + W1C : (e + 1) * WCOLS] = (
                w2q[g].reshape(RC, P, DIM).transpose(1, 0, 2).reshape(P, W2C)
            )
            for oc_i in range(OC):
                k = (e * OC + oc_i) * 2
                sb[:, k] = s2[g, oc_i * P : (oc_i + 1) * P]
                sb[:, k + 1] = b2[g, oc_i * P : (oc_i + 1) * P]
        in_maps.append({"xt": xt, "wg": wg, "sb": sb})
        pos_lists.append(core_pos)

    nc = _get_graph(C, use_fp8)
    res = run_bass_kernel_spmd(
        nc, in_maps, core_ids=list(range(NCORES)), trace=trace,
        trace_cores=trace_cores, **spmd_kwargs,
    )

    out_flat = np.zeros((B * K, DIM), np.float32)
    for core in range(NCORES):
        o = res.results[core]["out"]  # [P, EPC*OC*C]
        for e in range(EPC):
            pos = pos_lists[core][e]
            n = len(pos)
            if n == 0:
                continue
            blk = np.empty((n, DIM), np.float32)
            for oc_i in range(OC):
                cols = o[:, (e * OC + oc_i) * C : (e * OC + oc_i) * C + n]
                blk[:, oc_i * P : (oc_i + 1) * P] = cols.T
            out_flat[pos] = blk
    return out_flat.reshape(B, K, DIM), res


def kernel(**inputs) -> np.ndarray:
    out, _ = _run(inputs)
    return out


# revision 3
# speedup vs baseline: 1.9267x; 1.9267x over previous
"""Trainium2 Bass kernel for nn_AdaMLP (MoE routing, 64 experts, 2-layer MLP).

Strategy: expert-parallel over 8 NeuronCores; core i owns experts
[8i, 8i+8). The host groups slots by expert (the MoE dispatch), pads
each group to capacity C, and ships per core:
  - the 8 experts' weights quantized to fp8 e3m4 with per-output-channel
    scales (layer-1 scales folded into layer-2 weights, layer-2 scales
    applied on the PSUM->SBUF output op), clip factor per channel chosen
    to minimize weight MSE,
  - transposed slot groups xT in fp16,
  - per-expert output scale/bias columns in f32.
Each core computes, per expert:  H^T = relu(W1q^T-blocks @ xT),
Out^T = s2 * (W2q-blocks @ H^T) + b2, with the fp8 weights as the
stationary matmul operand.  fp8 weights halve the dominant HBM weight
stream (8.4 MB -> 4.2 MB per core) vs bf16; rel err ~1.8e-2 vs the f32
reference (gate 2e-2), deterministic for a fixed input set.

Schedule (v2, from trace analysis of the merged-DMA baseline): ALL input
DMAs ride the Sync HWDGE ring in exact consumption order:
  xt, sb, [w1(e0)], [w2(e0)|w1(e1)], ..., [w2(e6)|w1(e7)], [w2(e7)]
The pair-merged weight DMAs mean L1(e) is gated only on bytes it
actually needs (w1(e) is the tail of its pair), unlike the baseline's
per-expert [w1|w2] merge where L1(e0) waited ~1.3us for w2(e0) bytes.
xt/sb at the ring head land during the DMA pipeline fill, so the first
matmul starts as soon as w1(e0) lands. 10 issues x ~0.4-0.7us
descriptor-gen on Sync stay ahead of the ~1.1us/512KB data pitch.
The expert loop is software-pipelined via tile_wait_until floors so the
in-order PE runs L1(e+1) while Vector relus expert e. Vector does only
relus; Scalar does the dequant Copies. The final expert's dequant+store
is split per-oc-chunk across engines (oc0: Scalar ACTIVATE + Sync
store; oc1: Vector mult + Scalar store) so the two halves overlap.
Remaining exec time is dominated by fixed NEFF protocol: ~5.3us
preamble (start barriers + iram loads) and ~7us teardown (codegen's
unconditional 256-semaphore reset storm, ~51 resets/engine, PE slowest
at ~115 cyc each) around ~13us of streaming body.
"""

import numpy as np

P = 128                    # SBUF partitions
DIM = 256                  # slot dim
R = 1024                   # hidden dim
E = 64                     # num experts
NCORES = 8
EPC = E // NCORES          # experts per core
DC = DIM // P              # layer-1 contraction chunks (2)
RC = R // P                # r chunks (8)
OC = DIM // P              # output dim chunks (2)
W1C = DC * R               # w1 columns per expert (2048)
W2C = RC * DIM             # w2 columns per expert (2048)
WCOLS = W1C + W2C          # weight columns per expert (4096)

# fp8 e3m4 weight storage roughly halves the (dominant) weight-table DMA
# traffic vs bf16; measured rel err ~1.8e-2 vs the f32 reference (inside
# the 2e-2 gate). Set False for the bf16 fallback (~3.3e-3).
USE_FP8 = True
SHRINK_SEMS = True

_GRAPH_CACHE: dict = {}


def _build_graph(C: int, use_fp8: bool):
    import concourse.bacc as bacc
    import concourse.bass as bass_mod
    import concourse.tile as tile
    from concourse import mybir

    # Shrink the kernel semaphore range (the walrus codegen epilogue's
    # 256-sem reset storm is NOT affected by this — it is unconditional —
    # but a compact kernel range keeps sem allocation stable).
    if SHRINK_SEMS:
        bass_mod.get_kernel_semaphore_range = lambda: range(150, 198)
        import concourse.bass_utils as bu

        if not getattr(bu.get_walrus_args, "_max_sem_patch", False):
            orig_gwa = bu.get_walrus_args

            def _gwa(*a, **kw):
                return [*orig_gwa(*a, **kw), "--max-sem-num=198"]

            _gwa._max_sem_patch = True
            bu.get_walrus_args = _gwa

    f32 = mybir.dt.float32
    wdt = mybir.dt.float8e3 if use_fp8 else mybir.dt.bfloat16
    xdt = mybir.dt.float16 if use_fp8 else mybir.dt.bfloat16

    mx = mybir.AluOpType.max
    mm = mybir.AluOpType.mult

    nc = bacc.Bacc(None, target_bir_lowering=False)
    xt_ext = nc.declare_dram_parameter("xt", [P, DC * EPC * C], xdt, isOutput=False)
    # flat weight layout: per partition row, [w1(e0)|w2(e0)|w1(e1)|...]
    wg_ext = nc.declare_dram_parameter("wg", [P, EPC * WCOLS], wdt, isOutput=False)
    # per-expert output scale+bias columns: [s2 | b2] per oc chunk
    sb_ext = nc.declare_dram_parameter("sb", [P, EPC * OC * 2], f32, isOutput=False)
    out_ext = nc.declare_dram_parameter("out", [P, EPC * OC * C], f32, isOutput=True)

    with tile.TileContext(nc) as tc:
        with (
            tc.tile_pool(name="xpool", bufs=1) as xpool,
            tc.tile_pool(name="wpool", bufs=EPC + 1) as wpool,
            tc.tile_pool(name="hpool", bufs=4) as hpool,
            tc.tile_pool(name="opool", bufs=1) as opool,
            tc.tile_pool(name="ps1pool", bufs=3, space="PSUM") as ps1pool,
            tc.tile_pool(name="ps2pool", bufs=5, space="PSUM") as ps2pool,
        ):
            # The Sync HWDGE ring carries every input in consumption order.
            # Rings complete in order, so each DMA's semaphore gates exactly
            # the bytes scheduled before it; descriptor-gen (~0.4-0.7us per
            # issue) stays ahead of the ~1.1us/512KB data pitch.
            xt = xpool.tile([P, DC * EPC * C], xdt)
            nc.sync.dma_start(xt[:], xt_ext[:])
            sb = xpool.tile([P, EPC * OC * 2], f32)
            nc.sync.dma_start(sb[:], sb_ext[:])

            # Pair-merged weight DMAs with per-region tiles:
            #   wtile[0]   = w1(e0)                 cols [0, 2048)
            #   wtile[k]   = [w2(e[k-1]) | w1(e[k])] cols [4096k-2048, 4096k+2048)
            #   wtile[EPC] = w2(e7)                 cols [30720, 32768)
            # L1(e) reads the tail half of wtile[e]; L2(e) reads the head
            # half of wtile[e+1]; every matmul block lies in exactly one
            # tile, so Tile-framework deps gate on the right DMA.
            wtiles = []
            bounds = [0] + [k * WCOLS + W1C for k in range(EPC)] + [EPC * WCOLS]
            for k in range(EPC + 1):
                c0, c1 = bounds[k], bounds[k + 1]
                wt = wpool.tile([P, c1 - c0], wdt)
                nc.sync.dma_start(wt[:], wg_ext[:, c0:c1])
                wtiles.append(wt)

            def w1_slice(e, dc_i, rc_i):
                # w1(e) block (dc_i, rc_i) lives in wtile[e]; for e==0 the
                # tile starts at col 0, else at its second half (offset W2C).
                off = 0 if e == 0 else W2C
                c = off + dc_i * R + rc_i * P
                return wtiles[e][:, c : c + P]

            def w2_slice(e, rc_i, oc_i):
                # w2(e) block lives at the head of wtile[e+1].
                c = rc_i * DIM + oc_i * P
                return wtiles[e + 1][:, c : c + P]

            # single output staging tile; experts 0..6 stored in one bulk
            # DMA during expert 7's compute, expert 7 split per-oc across
            # Sync and Scalar so dequant and store of the halves overlap.
            out_sb = opool.tile([P, EPC * OC * C], f32)

            hs = {}

            def layer2(e):
                # layer 2: Out^T[dim,:] = sum_r W2[r, dim-block] . H^T[r, :]
                h = hs.pop(e)
                ps2 = ps2pool.tile([P, OC * C], f32)
                for oc_i in range(OC):
                    for rc_i in range(RC):
                        nc.tensor.matmul(
                            ps2[:, oc_i * C : oc_i * C + C],
                            w2_slice(e, rc_i, oc_i),
                            h[:, rc_i * C : rc_i * C + C],
                            start=(rc_i == 0),
                            stop=(rc_i == RC - 1),
                        )
                # dequant scale on Scalar only (activation Copy with
                # per-partition scale; b2 == 0, checked on host): Vector
                # must stay relu-only — giving it out-ops chains relu(e)
                # behind L2(e-1) through its in-order queue. GPSIMD can't
                # read PSUM.
                base = e * OC * C
                for oc_i in range(OC):
                    if e == EPC - 1 and oc_i == 1:
                        # last expert only: run oc1 on Vector in parallel
                        # with oc0 on Scalar — no later relu exists for this
                        # to block, and it halves the final dequant latency.
                        nc.vector.tensor_scalar(
                            out_sb[:, base + C : base + 2 * C],
                            ps2[:, C : 2 * C],
                            sb[:, (e * OC + 1) * 2 : (e * OC + 1) * 2 + 1],
                            None,
                            mm,
                        )
                        continue
                    nc.scalar.activation(
                        out_sb[:, base + oc_i * C : base + (oc_i + 1) * C],
                        ps2[:, oc_i * C : oc_i * C + C],
                        mybir.ActivationFunctionType.Copy,
                        bias=0.0,
                        scale=sb[:, (e * OC + oc_i) * 2 : (e * OC + oc_i) * 2 + 1],
                    )
                if e == EPC - 2:
                    # store experts 0..6 while expert 7 computes; only
                    # e7's store rides the critical tail.
                    nc.scalar.dma_start(
                        out_ext[:, : (EPC - 1) * OC * C],
                        out_sb[:, : (EPC - 1) * OC * C],
                    )
                if e == EPC - 1:
                    # split the final store: oc0 (Scalar ACTIVATE result)
                    # goes out on Sync — idle by now — while Vector still
                    # dequants oc1; oc1 follows on Scalar. Two ~350-cycle
                    # descriptor-gens in parallel instead of one ~650.
                    nc.sync.dma_start(
                        out_ext[:, base : base + C], out_sb[:, base : base + C]
                    )
                    nc.scalar.dma_start(
                        out_ext[:, base + C : base + 2 * C],
                        out_sb[:, base + C : base + 2 * C],
                    )

            # Software pipeline: the PE queue is in-order, so L1(e) must be
            # SCHEDULED before L2(e-1) — the PE then runs L1(e) while Vector
            # does relu(e-1) instead of stalling. Emission order alone does
            # not guarantee this (the Tile scheduler re-simulates and its
            # DMA model makes the weight arrivals look later than they
            # land), so pace the schedule explicitly: L1(e) floored at the
            # stream cadence, L2(e-1) floored just after L1(e).
            for e in range(EPC):
                with tc.tile_wait_until(0.010 + 0.003 * e):
                    # layer 1: H^T[r,:] = sum_d W1[d, r-block] . xT[d, :]
                    # 8 accumulation groups at offsets of one PSUM tile.
                    ps1 = ps1pool.tile([P, RC * C], f32)
                    for rc_i in range(RC):
                        for dc_i in range(DC):
                            nc.tensor.matmul(
                                ps1[:, rc_i * C : rc_i * C + C],
                                w1_slice(e, dc_i, rc_i),
                                xt[:, (dc_i * EPC + e) * C : (dc_i * EPC + e) * C + C],
                                start=(dc_i == 0),
                                stop=(dc_i == DC - 1),
                            )
                    # single fused relu over all 8 chunks (b1 == 0; checked
                    # on host), on Vector; Vector does nothing else.
                    h = hpool.tile([P, RC * C], xdt)
                    nc.vector.tensor_scalar(h[:], ps1[:], 0.0, None, mx)
                    hs[e] = h
                if e >= 1:
                    with tc.tile_wait_until(0.011 + 0.003 * e):
                        layer2(e - 1)
            with tc.tile_wait_until(0.011 + 0.003 * EPC):
                layer2(EPC - 1)
    nc.compile()
    return nc


def _get_graph(C: int, use_fp8: bool):
    key = (C, use_fp8)
    if key not in _GRAPH_CACHE:
        _GRAPH_CACHE[key] = _build_graph(C, use_fp8)
    return _GRAPH_CACHE[key]


def _quant_e3m4_chan(w, np_e3m4):
    """Quantize w [n_chan along last axis] to e3m4 with per-channel scale;
    clip factor per channel picked from a small grid to minimize MSE.
    w: (..., K, N) quantized per-column-N over axis -2. Returns (q, s)."""
    amax = np.abs(w).max(axis=-2, keepdims=True)
    amax = np.maximum(amax, 1e-30)
    best_err = None
    best_q = None
    best_s = None
    for g in (1.0, 1.05, 1.1, 1.2, 1.35, 1.5):
        s = amax * (g / 15.5)
        q = np.clip(w / s, -15.5, 15.5).astype(np_e3m4)
        err = ((q.astype(np.float32) * s - w) ** 2).sum(axis=-2, keepdims=True)
        if best_err is None:
            best_err, best_q, best_s = err, q, s
        else:
            m = err < best_err
            best_err = np.where(m, err, best_err)
            best_q = np.where(np.broadcast_to(m, q.shape), q, best_q)
            best_s = np.where(m, s, best_s)
    return best_q, best_s[..., 0, :]


def _run(inputs: dict, trace: bool = False, trace_cores=None, use_bf16=None,
         use_fp8=None, **spmd_kwargs):
    from concourse.bass_utils import run_bass_kernel_spmd
    import ml_dtypes

    if use_fp8 is None:
        use_fp8 = USE_FP8 and not use_bf16

    if use_fp8:
        wdt_np = ml_dtypes.float8_e3m4
        xdt_np = np.float16
    else:
        wdt_np = ml_dtypes.bfloat16
        xdt_np = ml_dtypes.bfloat16

    slots = np.asarray(inputs["slots"], np.float32)
    w1 = np.asarray(inputs["w1"], np.float32)
    b1 = np.asarray(inputs["b1"], np.float32)
    w2 = np.asarray(inputs["w2"], np.float32)
    b2 = np.asarray(inputs["b2"], np.float32)
    indices = np.asarray(inputs["indices"]).astype(np.int64)

    B, K, D = slots.shape
    assert D == DIM and w1.shape == (E, DIM, R) and w2.shape == (E, R, DIM)
    assert not b1.any(), "nonzero b1 needs the per-chunk bias path"
    assert not b2.any(), "nonzero b2 needs the tensor_scalar output path"
    X = slots.reshape(B * K, DIM)
    idx = indices.reshape(B * K)

    counts = np.bincount(idx, minlength=E)
    C = max(int(counts.max()), 16)
    C = ((C + 15) // 16) * 16  # stable capacities -> stable NEFF cache keys

    if use_fp8:
        # per-channel-r scales for w1; fold s1 into w2 rows; per-channel-d
        # scales for w2 applied on-device via the output tensor_scalar.
        w1q, s1 = _quant_e3m4_chan(w1, wdt_np)          # (E,D,R), (E,R)
        w2p = w2 * s1[:, :, None]
        w2q, s2 = _quant_e3m4_chan(w2p, wdt_np)          # (E,R,D), (E,D)
    else:
        w1q = w1.astype(wdt_np)
        w2q = w2.astype(wdt_np)
        s2 = np.ones((E, DIM), np.float32)

    in_maps = []
    pos_lists = []
    for core in range(NCORES):
        xt = np.zeros((P, DC * EPC * C), xdt_np)
        wg = np.empty((P, EPC * WCOLS), wdt_np)
        sb = np.zeros((P, EPC * OC * 2), np.float32)
        core_pos = []
        for e in range(EPC):
            g = core * EPC + e
            pos = np.nonzero(idx == g)[0]
            core_pos.append(pos)
            n = len(pos)
            if n:
                xeT = X[pos].T.astype(xdt_np)  # [DIM, n]
                for dc_i in range(DC):
                    xt[:, (dc_i * EPC + e) * C : (dc_i * EPC + e) * C + n] = (
                        xeT[dc_i * P : (dc_i + 1) * P]
                    )
            wg[:, e * WCOLS : e * WCOLS + W1C] = (
                w1q[g].reshape(DC, P, R).transpose(1, 0, 2).reshape(P, W1C)
            )
            wg[:, e * WCOLS + W1C : (e + 1) * WCOLS] = (
                w2q[g].reshape(RC, P, DIM).transpose(1, 0, 2).reshape(P, W2C)
            )
            for oc_i in range(OC):
                k = (e * OC + oc_i) * 2
                sb[:, k] = s2[g, oc_i * P : (oc_i + 1) * P]
                sb[:, k + 1] = b2[g, oc_i * P : (oc_i + 1) * P]
        in_maps.append({"xt": xt, "wg": wg, "sb": sb})
        pos_lists.append(core_pos)

    nc = _get_graph(C, use_fp8)
    res = run_bass_kernel_spmd(
        nc, in_maps, core_ids=list(range(NCORES)), trace=trace,
        trace_cores=trace_cores, **spmd_kwargs,
    )

    out_flat = np.zeros((B * K, DIM), np.float32)
    for core in range(NCORES):
        o = res.results[core]["out"]  # [P, EPC*OC*C]
        for e in range(EPC):
            pos = pos_lists[core][e]
            n = len(pos)
            if n == 0:
                continue
            blk = np.empty((n, DIM), np.float32)
            for oc_i in range(OC):
                cols = o[:, (e * OC + oc_i) * C : (e * OC + oc_i) * C + n]
                blk[:, oc_i * P : (oc_i + 1) * P] = cols.T
            out_flat[pos] = blk
    return out_flat.reshape(B, K, DIM), res


def kernel(**inputs) -> np.ndarray:
    out, _ = _run(inputs)
    return out


# revision 4
# speedup vs baseline: 1.9503x; 1.0122x over previous
"""Trainium2 Bass kernel for nn_AdaMLP (MoE routing, 64 experts, 2-layer MLP).

Strategy: expert-parallel over 8 NeuronCores; core i owns experts
[8i, 8i+8). The host groups slots by expert (the MoE dispatch), pads
each group to capacity C, and ships per core:
  - the 8 experts' weights quantized to fp8 e3m4 with per-output-channel
    scales (layer-1 scales folded into layer-2 weights, layer-2 scales
    applied on the PSUM->SBUF output op), clip factor per channel chosen
    to minimize weight MSE,
  - transposed slot groups xT in fp16,
  - per-expert output scale/bias columns in f32.
Each core computes, per expert:  H^T = relu(W1q^T-blocks @ xT),
Out^T = s2 * (W2q-blocks @ H^T) + b2, with the fp8 weights as the
stationary matmul operand.  fp8 weights halve the dominant HBM weight
stream (8.4 MB -> 4.2 MB per core) vs bf16; rel err ~1.8e-2 vs the f32
reference (gate 2e-2), deterministic for a fixed input set.

Schedule (v2, from trace analysis of the merged-DMA baseline): ALL input
DMAs ride the Sync HWDGE ring in exact consumption order:
  xt, sb, [w1(e0)], [w2(e0)|w1(e1)], ..., [w2(e6)|w1(e7)], [w2(e7)]
The pair-merged weight DMAs mean L1(e) is gated only on bytes it
actually needs (w1(e) is the tail of its pair), unlike the baseline's
per-expert [w1|w2] merge where L1(e0) waited ~1.3us for w2(e0) bytes.
xt/sb at the ring head land during the DMA pipeline fill, so the first
matmul starts as soon as w1(e0) lands. 10 issues x ~0.4-0.7us
descriptor-gen on Sync stay ahead of the ~1.1us/512KB data pitch.
The expert loop is software-pipelined via tile_wait_until floors so the
in-order PE runs L1(e+1) while Vector relus expert e. Vector does only
relus; Scalar does the dequant Copies. The final expert's dequant+store
is split per-oc-chunk across engines (oc0: Scalar ACTIVATE + Sync
store; oc1: Vector mult + Scalar store) so the two halves overlap.
Remaining exec time is dominated by fixed NEFF protocol: ~5.3us
preamble (start barriers + iram loads) and ~7us teardown (codegen's
unconditional 256-semaphore reset storm, ~51 resets/engine, PE slowest
at ~115 cyc each) around ~13us of streaming body.
"""

import numpy as np

P = 128                    # SBUF partitions
DIM = 256                  # slot dim
R = 1024                   # hidden dim
E = 64                     # num experts
NCORES = 8
EPC = E // NCORES          # experts per core
DC = DIM // P              # layer-1 contraction chunks (2)
RC = R // P                # r chunks (8)
OC = DIM // P              # output dim chunks (2)
W1C = DC * R               # w1 columns per expert (2048)
W2C = RC * DIM             # w2 columns per expert (2048)
WCOLS = W1C + W2C          # weight columns per expert (4096)

# fp8 e3m4 weight storage roughly halves the (dominant) weight-table DMA
# traffic vs bf16; measured rel err ~1.8e-2 vs the f32 reference (inside
# the 2e-2 gate). Set False for the bf16 fallback (~3.3e-3).
USE_FP8 = True
SHRINK_SEMS = True

_GRAPH_CACHE: dict = {}


def _build_graph(C: int, use_fp8: bool):
    import concourse.bacc as bacc
    import concourse.bass as bass_mod
    import concourse.tile as tile
    from concourse import mybir

    # Shrink the kernel semaphore range (the walrus codegen epilogue's
    # 256-sem reset storm is NOT affected by this — it is unconditional —
    # but a compact kernel range keeps sem allocation stable).
    if SHRINK_SEMS:
        bass_mod.get_kernel_semaphore_range = lambda: range(150, 198)
        import concourse.bass_utils as bu

        if not getattr(bu.get_walrus_args, "_max_sem_patch", False):
            orig_gwa = bu.get_walrus_args

            def _gwa(*a, **kw):
                return [*orig_gwa(*a, **kw), "--max-sem-num=198"]

            _gwa._max_sem_patch = True
            bu.get_walrus_args = _gwa

    f32 = mybir.dt.float32
    wdt = mybir.dt.float8e3 if use_fp8 else mybir.dt.bfloat16
    xdt = mybir.dt.float16 if use_fp8 else mybir.dt.bfloat16

    mx = mybir.AluOpType.max
    mm = mybir.AluOpType.mult

    nc = bacc.Bacc(None, target_bir_lowering=False)
    xt_ext = nc.declare_dram_parameter("xt", [P, DC * EPC * C], xdt, isOutput=False)
    # flat weight layout: per partition row, [w1(e0)|w2(e0)|w1(e1)|...]
    wg_ext = nc.declare_dram_parameter("wg", [P, EPC * WCOLS], wdt, isOutput=False)
    # per-expert output scale+bias columns: [s2 | b2] per oc chunk
    sb_ext = nc.declare_dram_parameter("sb", [P, EPC * OC * 2], f32, isOutput=False)
    out_ext = nc.declare_dram_parameter("out", [P, EPC * OC * C], f32, isOutput=True)

    with tile.TileContext(nc) as tc:
        with (
            tc.tile_pool(name="xpool", bufs=1) as xpool,
            tc.tile_pool(name="wpool", bufs=EPC + 1) as wpool,
            tc.tile_pool(name="hpool", bufs=4) as hpool,
            tc.tile_pool(name="opool", bufs=1) as opool,
            tc.tile_pool(name="ps1pool", bufs=3, space="PSUM") as ps1pool,
            tc.tile_pool(name="ps2pool", bufs=5, space="PSUM") as ps2pool,
        ):
            # The Sync HWDGE ring carries every input in consumption order.
            # Rings complete in order, so each DMA's semaphore gates exactly
            # the bytes scheduled before it; descriptor-gen (~0.4-0.7us per
            # issue) stays ahead of the ~1.1us/512KB data pitch.
            xt = xpool.tile([P, DC * EPC * C], xdt)
            nc.sync.dma_start(xt[:], xt_ext[:])
            sb = xpool.tile([P, EPC * OC * 2], f32)

            # Pair-merged weight DMAs with per-region tiles:
            #   wtile[0]   = w1(e0)                 cols [0, 2048)
            #   wtile[k]   = [w2(e[k-1]) | w1(e[k])] cols [4096k-2048, 4096k+2048)
            #   wtile[EPC] = w2(e7)                 cols [30720, 32768)
            # L1(e) reads the tail half of wtile[e]; L2(e) reads the head
            # half of wtile[e+1]; every matmul block lies in exactly one
            # tile, so Tile-framework deps gate on the right DMA.
            # sb (16 KB in 128-byte packets — slow per-packet) rides after
            # A1: off the ramp-critical ring head, but still ~4 us before
            # the first dequant needs it.
            wtiles = []
            bounds = [0] + [k * WCOLS + W1C for k in range(EPC)] + [EPC * WCOLS]
            for k in range(EPC + 1):
                c0, c1 = bounds[k], bounds[k + 1]
                wt = wpool.tile([P, c1 - c0], wdt)
                nc.sync.dma_start(wt[:], wg_ext[:, c0:c1])
                wtiles.append(wt)
                if k == 1:
                    nc.sync.dma_start(sb[:], sb_ext[:])

            def w1_slice(e, dc_i, rc_i):
                # w1(e) block (dc_i, rc_i) lives in wtile[e]; for e==0 the
                # tile starts at col 0, else at its second half (offset W2C).
                off = 0 if e == 0 else W2C
                c = off + dc_i * R + rc_i * P
                return wtiles[e][:, c : c + P]

            def w2_slice(e, rc_i, oc_i):
                # w2(e) block lives at the head of wtile[e+1].
                c = rc_i * DIM + oc_i * P
                return wtiles[e + 1][:, c : c + P]

            # single output staging tile; experts 0..6 stored in one bulk
            # DMA during expert 7's compute, expert 7 split per-oc across
            # Sync and Scalar so dequant and store of the halves overlap.
            out_sb = opool.tile([P, EPC * OC * C], f32)

            hs = {}

            def layer2(e):
                # layer 2: Out^T[dim,:] = sum_r W2[r, dim-block] . H^T[r, :]
                h = hs.pop(e)
                ps2 = ps2pool.tile([P, OC * C], f32)
                for oc_i in range(OC):
                    for rc_i in range(RC):
                        nc.tensor.matmul(
                            ps2[:, oc_i * C : oc_i * C + C],
                            w2_slice(e, rc_i, oc_i),
                            h[:, rc_i * C : rc_i * C + C],
                            start=(rc_i == 0),
                            stop=(rc_i == RC - 1),
                        )
                # dequant scale on Scalar only (activation Copy with
                # per-partition scale; b2 == 0, checked on host): Vector
                # must stay relu-only — giving it out-ops chains relu(e)
                # behind L2(e-1) through its in-order queue. GPSIMD can't
                # read PSUM.
                base = e * OC * C
                for oc_i in range(OC):
                    if e == EPC - 1 and oc_i == 1:
                        # last expert only: run oc1 on Vector in parallel
                        # with oc0 on Scalar — no later relu exists for this
                        # to block, and it halves the final dequant latency.
                        nc.vector.tensor_scalar(
                            out_sb[:, base + C : base + 2 * C],
                            ps2[:, C : 2 * C],
                            sb[:, (e * OC + 1) * 2 : (e * OC + 1) * 2 + 1],
                            None,
                            mm,
                        )
                        continue
                    nc.scalar.activation(
                        out_sb[:, base + oc_i * C : base + (oc_i + 1) * C],
                        ps2[:, oc_i * C : oc_i * C + C],
                        mybir.ActivationFunctionType.Copy,
                        bias=0.0,
                        scale=sb[:, (e * OC + oc_i) * 2 : (e * OC + oc_i) * 2 + 1],
                    )
                if e == EPC - 2:
                    # store experts 0..6 while expert 7 computes; only
                    # e7's store rides the critical tail.
                    nc.scalar.dma_start(
                        out_ext[:, : (EPC - 1) * OC * C],
                        out_sb[:, : (EPC - 1) * OC * C],
                    )
                if e == EPC - 1:
                    # split the final store: oc0 (Scalar ACTIVATE result)
                    # goes out on Sync — idle by now — while Vector still
                    # dequants oc1; oc1 follows on Scalar. Two ~350-cycle
                    # descriptor-gens in parallel instead of one ~650.
                    nc.sync.dma_start(
                        out_ext[:, base : base + C], out_sb[:, base : base + C]
                    )
                    nc.scalar.dma_start(
                        out_ext[:, base + C : base + 2 * C],
                        out_sb[:, base + C : base + 2 * C],
                    )

            # Software pipeline: the PE queue is in-order, so L1(e) must be
            # SCHEDULED before L2(e-1) — the PE then runs L1(e) while Vector
            # does relu(e-1) instead of stalling. Emission order alone does
            # not guarantee this (the Tile scheduler re-simulates and its
            # DMA model makes the weight arrivals look later than they
            # land), so pace the schedule explicitly: L1(e) floored at the
            # stream cadence, L2(e-1) floored just after L1(e).
            for e in range(EPC):
                with tc.tile_wait_until(0.010 + 0.003 * e):
                    # layer 1: H^T[r,:] = sum_d W1[d, r-block] . xT[d, :]
                    # 8 accumulation groups at offsets of one PSUM tile.
                    ps1 = ps1pool.tile([P, RC * C], f32)
                    for rc_i in range(RC):
                        for dc_i in range(DC):
                            nc.tensor.matmul(
                                ps1[:, rc_i * C : rc_i * C + C],
                                w1_slice(e, dc_i, rc_i),
                                xt[:, (dc_i * EPC + e) * C : (dc_i * EPC + e) * C + C],
                                start=(dc_i == 0),
                                stop=(dc_i == DC - 1),
                            )
                    # single fused relu over all 8 chunks (b1 == 0; checked
                    # on host), on Vector; Vector does nothing else.
                    h = hpool.tile([P, RC * C], xdt)
                    nc.vector.tensor_scalar(h[:], ps1[:], 0.0, None, mx)
                    hs[e] = h
                if e >= 1:
                    with tc.tile_wait_until(0.011 + 0.003 * e):
                        layer2(e - 1)
            with tc.tile_wait_until(0.011 + 0.003 * EPC):
                layer2(EPC - 1)
    nc.compile()
    return nc


def _get_graph(C: int, use_fp8: bool):
    key = (C, use_fp8)
    if key not in _GRAPH_CACHE:
        _GRAPH_CACHE[key] = _build_graph(C, use_fp8)
    return _GRAPH_CACHE[key]


def _quant_e3m4_chan(w, np_e3m4):
    """Quantize w [n_chan along last axis] to e3m4 with per-channel scale;
    clip factor per channel picked from a small grid to minimize MSE.
    w: (..., K, N) quantized per-column-N over axis -2. Returns (q, s)."""
    amax = np.abs(w).max(axis=-2, keepdims=True)
    amax = np.maximum(amax, 1e-30)
    best_err = None
    best_q = None
    best_s = None
    for g in (1.0, 1.05, 1.1, 1.2, 1.35, 1.5):
        s = amax * (g / 15.5)
        q = np.clip(w / s, -15.5, 15.5).astype(np_e3m4)
        err = ((q.astype(np.float32) * s - w) ** 2).sum(axis=-2, keepdims=True)
        if best_err is None:
            best_err, best_q, best_s = err, q, s
        else:
            m = err < best_err
            best_err = np.where(m, err, best_err)
            best_q = np.where(np.broadcast_to(m, q.shape), q, best_q)
            best_s = np.where(m, s, best_s)
    return best_q, best_s[..., 0, :]


def _run(inputs: dict, trace: bool = False, trace_cores=None, use_bf16=None,
         use_fp8=None, **spmd_kwargs):
    from concourse.bass_utils import run_bass_kernel_spmd
    import ml_dtypes

    if use_fp8 is None:
        use_fp8 = USE_FP8 and not use_bf16

    if use_fp8:
        wdt_np = ml_dtypes.float8_e3m4
        xdt_np = np.float16
    else:
        wdt_np = ml_dtypes.bfloat16
        xdt_np = ml_dtypes.bfloat16

    slots = np.asarray(inputs["slots"], np.float32)
    w1 = np.asarray(inputs["w1"], np.float32)
    b1 = np.asarray(inputs["b1"], np.float32)
    w2 = np.asarray(inputs["w2"], np.float32)
    b2 = np.asarray(inputs["b2"], np.float32)
    indices = np.asarray(inputs["indices"]).astype(np.int64)

    B, K, D = slots.shape
    assert D == DIM and w1.shape == (E, DIM, R) and w2.shape == (E, R, DIM)
    assert not b1.any(), "nonzero b1 needs the per-chunk bias path"
    assert not b2.any(), "nonzero b2 needs the tensor_scalar output path"
    X = slots.reshape(B * K, DIM)
    idx = indices.reshape(B * K)

    counts = np.bincount(idx, minlength=E)
    C = max(int(counts.max()), 16)
    C = ((C + 15) // 16) * 16  # stable capacities -> stable NEFF cache keys

    if use_fp8:
        # per-channel-r scales for w1; fold s1 into w2 rows; per-channel-d
        # scales for w2 applied on-device via the output tensor_scalar.
        w1q, s1 = _quant_e3m4_chan(w1, wdt_np)          # (E,D,R), (E,R)
        w2p = w2 * s1[:, :, None]
        w2q, s2 = _quant_e3m4_chan(w2p, wdt_np)          # (E,R,D), (E,D)
    else:
        w1q = w1.astype(wdt_np)
        w2q = w2.astype(wdt_np)
        s2 = np.ones((E, DIM), np.float32)

    in_maps = []
    pos_lists = []
    for core in range(NCORES):
        xt = np.zeros((P, DC * EPC * C), xdt_np)
        wg = np.empty((P, EPC * WCOLS), wdt_np)
        sb = np.zeros((P, EPC * OC * 2), np.float32)
        core_pos = []
        for e in range(EPC):
            g = core * EPC + e
            pos = np.nonzero(idx == g)[0]
            core_pos.append(pos)
            n = len(pos)
            if n:
                xeT = X[pos].T.astype(xdt_np)  # [DIM, n]
                for dc_i in range(DC):
                    xt[:, (dc_i * EPC + e) * C : (dc_i * EPC + e) * C + n] = (
                        xeT[dc_i * P : (dc_i + 1) * P]
                    )
            wg[:, e * WCOLS : e * WCOLS + W1C] = (
                w1q[g].reshape(DC, P, R).transpose(1, 0, 2).reshape(P, W1C)
            )
            wg[:, e * WCOLS + W1C : (e + 1) * WCOLS] = (
                w2q[g].reshape(RC, P, DIM).transpose(1, 0, 2).reshape(P, W2C)
            )
            for oc_i in range(OC):
                k = (e * OC + oc_i) * 2
                sb[:, k] = s2[g, oc_i * P : (oc_i + 1) * P]
                sb[:, k + 1] = b2[g, oc_i * P : (oc_i + 1) * P]
        in_maps.append({"xt": xt, "wg": wg, "sb": sb})
        pos_lists.append(core_pos)

    nc = _get_graph(C, use_fp8)
    res = run_bass_kernel_spmd(
        nc, in_maps, core_ids=list(range(NCORES)), trace=trace,
        trace_cores=trace_cores, **spmd_kwargs,
    )

    out_flat = np.zeros((B * K, DIM), np.float32)
    for core in range(NCORES):
        o = res.results[core]["out"]  # [P, EPC*OC*C]
        for e in range(EPC):
            pos = pos_lists[core][e]
            n = len(pos)
            if n == 0:
                continue
            blk = np.empty((n, DIM), np.float32)
            for oc_i in range(OC):
                cols = o[:, (e * OC + oc_i) * C : (e * OC + oc_i) * C + n]
                blk[:, oc_i * P : (oc_i + 1) * P] = cols.T
            out_flat[pos] = blk
    return out_flat.reshape(B, K, DIM), res


def kernel(**inputs) -> np.ndarray:
    out, _ = _run(inputs)
    return out
